# revision 6
# baseline (speedup 1.0000x reference)
"""Trainium2 Bass kernel for CustomExtractorSNN — PSUM-state architecture.

The LIF recurrence M_{t+1} = beta*M_t + c - [M_t>1] is tracked in PSUM as
X_t = beta^(1-t) * (M_t - 1) = X̂_t (tensor part) + K_t (per-partition const).
 - compares: Act engine Sign(X̂ + K_t) -> sigma_t in {-1, +1}
 - spike injection: PE diag-matmul accumulate  X̂ += diag(-c_t/2) @ sigma_t
 - current injection: DVE STT in-place         X̂ += c_t * q      (exact f32)
 - X-init (q = x @ (W1/thr).T): fp16-pair matmuls (x = xh+xl, W = Wh+Wl)
The lif2/lif3 tail continues the same linear form so tanh reads PSUM directly.
Critic path and outputs are fp16. Host folds pi into the actor upcast.
"""

import os
from contextlib import ExitStack

import numpy as np

import concourse.bass as bass
import concourse.tile as tile
from concourse import bacc, mybir
from concourse.bass_utils import run_bass_kernel_spmd

N_CORES = 8
B_FULL = 131072
F = 256
H = 64
B_CORE = B_FULL // N_CORES

FD = 1024
CHUNK = 2 * FD
TIMESTEPS = 10
PI = float(np.pi)
CBITS = 12  # f32r truncates operands to 12 mantissa bits (measured on HW)

f32 = mybir.dt.float32
f32r = mybir.dt.float32r
fp16 = mybir.dt.float16
Alu = mybir.AluOpType
Act = mybir.ActivationFunctionType

_BUILD_CACHE: dict = {}


def _build(bcore: int) -> bass.Bass:
    if bcore in _BUILD_CACHE:
        return _BUILD_CACHE[bcore]
    assert bcore % CHUNK == 0
    n_super = bcore // CHUNK

    nc = bacc.Bacc("TRN2", target_bir_lowering=False, debug=False, num_devices=N_CORES)

    xh_d = nc.dram_tensor("xh", [F, bcore], fp16, kind="ExternalInput")
    xl_d = nc.dram_tensor("xl", [F, bcore], fp16, kind="ExternalInput")
    # fp16 weight pack: w1h chunks [0:512], w1l [512:1024], wv1 [1024:1536], wv2bd [1536:1664]
    wp16 = nc.dram_tensor("wp16", [128, 1664], fp16, kind="ExternalInput")
    # f32r pack: sigma diag per step t=1..9 [t*128:(t+1)*128-...], acmb, bcmb
    wpr = nc.dram_tensor("wpr", [128, 11 * 128], f32r, kind="ExternalInput")
    # f32 vec tables: K_1..K_10 (0..9), K_11 (10), L (11), bias_f (12), bv1 (13), bv2 (14)
    vecs = nc.dram_tensor("vecs", [128, 16], f32, kind="ExternalInput")
    cvec = nc.dram_tensor("cvec", [128, 16], f32, kind="ExternalInput")  # c_1..c_9

    actorT = nc.dram_tensor("actorT", [H, bcore], fp16, kind="ExternalOutput")
    criticT = nc.dram_tensor("criticT", [H, bcore], fp16, kind="ExternalOutput")

    with tile.TileContext(nc) as tc, ExitStack() as ctx:
        wpool = ctx.enter_context(tc.tile_pool(name="weights", bufs=1))
        xpool = ctx.enter_context(tc.tile_pool(name="x", bufs=1))
        spool = ctx.enter_context(tc.tile_pool(name="sig", bufs=2))
        qpool = ctx.enter_context(tc.tile_pool(name="q", bufs=1))
        opool = ctx.enter_context(tc.tile_pool(name="outs", bufs=2))
        psX = ctx.enter_context(tc.tile_pool(name="psX", bufs=1, space=bass.MemorySpace.PSUM))
        psV = ctx.enter_context(tc.tile_pool(name="psV", bufs=1, space=bass.MemorySpace.PSUM))
        psC = ctx.enter_context(tc.tile_pool(name="psC", bufs=1, space=bass.MemorySpace.PSUM))

        w16 = wpool.tile([128, 1664], fp16, tag="wp16")
        nc.sync.dma_start(w16[:], wp16[:])
        wr = wpool.tile([128, 11 * 128], f32r, tag="wpr")
        nc.sync.dma_start(wr[:], wpr[:])
        vt = wpool.tile([128, 16], f32, tag="vecs")
        nc.sync.dma_start(vt[:], vecs[:])
        ct = wpool.tile([128, 16], f32, tag="cvec")
        nc.sync.dma_start(ct[:], cvec[:])

        w1h = [w16[:, k * 128:(k + 1) * 128] for k in range(4)]
        w1l = [w16[:, 512 + k * 128:512 + (k + 1) * 128] for k in range(4)]
        wv1 = [w16[:, 1024 + k * 128:1024 + (k + 1) * 128] for k in range(4)]
        wv2 = w16[:, 1536:1664]
        dsig = [wr[:, t * 128:(t + 1) * 128] for t in range(9)]
        acmb = wr[:, 9 * 128:10 * 128]
        bcmb = wr[:, 10 * 128:11 * 128]

        # interleave supertiles step-by-step; groups of 3 (6 X-banks), remainder smaller
        if n_super >= 3:
            sizes = []
            left = n_super
            while left >= 3:
                sizes.append(3)
                left -= 3
            if left:
                sizes.append(left)
        else:
            sizes = [n_super]

        def try_mm(out, lhsT, rhs, start, stop):
            # PSUM matmul outputs cannot span banks: split into 512-col halves
            for j in range(2):
                sl = slice(j * 512, (j + 1) * 512)
                nc.tensor.matmul(out[:, sl], lhsT, rhs[:, sl], start=start, stop=stop)

        base = 0
        for G in sizes:
            sts = [base + i for i in range(G)]
            base += G
            Xs, qs, a0s = [], [], []
            xhs = []
            for s in sts:
                a0 = s * CHUNK
                a0s.append(a0)
                xh = []
                xl = []
                for k in range(4):
                    th = xpool.tile([128, FD], fp16, tag=f"xh{k}_{s % 3}")
                    tl = xpool.tile([128, FD], fp16, tag=f"xl{k}_{s % 3}")
                    half = (k % 2) * 128
                    col = a0 + (k // 2) * FD
                    nc.sync.dma_start(th[:], xh_d[half:half + 128, col:col + FD])
                    nc.sync.dma_start(tl[:], xl_d[half:half + 128, col:col + FD])
                    xh.append(th)
                    xl.append(tl)
                xhs.append(xh)
                # ---- X init: q = x @ (W1/thr).T via fp16 pair (hh + hl + lh) ----
                X = psX.tile([128, FD], f32, tag=f"X{s % 3}")
                first = True
                for k in range(4):
                    try_mm(X[:], w1h[k], xh[k][:], first, False)
                    first = False
                    try_mm(X[:], w1h[k], xl[k][:], False, False)
                    try_mm(X[:], w1l[k], xh[k][:], False, k == 3)
                Xs.append(X)
                q = qpool.tile([128, FD], f32, tag=f"q{s % 3}")
                nc.scalar.activation(q[:], X[:], Act.Copy)
                qs.append(q)

            # ---- recurrence: 9 steps, supertiles interleaved ----
            for t in range(1, TIMESTEPS):
                sgs = []
                for i in range(G):
                    sg = spool.tile([128, FD], f32r, tag=f"sg{i}")
                    nc.scalar.activation(sg[:], Xs[i][:], Act.Sign,
                                         bias=vt[:, t - 1:t], scale=1.0)
                    sgs.append(sg)
                for i in range(G):
                    nc.vector.scalar_tensor_tensor(
                        Xs[i][:], qs[i][:], ct[:, t - 1:t], Xs[i][:],
                        Alu.mult, Alu.add)
                for i in range(G):
                    try_mm(Xs[i][:], dsig[t - 1], sgs[i][:], False, True)

            # ---- tail ----
            sg10s, sg2s = [], []
            for i in range(G):
                sg = spool.tile([128, FD], f32r, tag=f"sgA{i}")
                nc.scalar.activation(sg[:], Xs[i][:], Act.Sign, bias=vt[:, 9:10], scale=1.0)
                sg10s.append(sg)
            for i in range(G):
                try_mm(Xs[i][:], acmb, sg10s[i][:], False, True)
            for i in range(G):
                sg = spool.tile([128, FD], f32r, tag=f"sgB{i}")
                nc.scalar.activation(sg[:], Xs[i][:], Act.Sign, bias=vt[:, 10:11], scale=1.0)
                sg2s.append(sg)
            for i in range(G):
                try_mm(Xs[i][:], bcmb, sg2s[i][:], False, True)
            for i in range(G):
                act = opool.tile([128, FD], fp16, tag=f"act{i}")
                nc.scalar.activation(act[:], Xs[i][:], Act.Tanh, bias=vt[:, 12:13],
                                     scale=vt[:, 11:12])
                nc.sync.dma_start(actorT[:, a0s[i]:a0s[i] + FD], act[0:64, :])
                nc.sync.dma_start(actorT[:, a0s[i] + FD:a0s[i] + CHUNK], act[64:128, :])

            # ---- critic path (fp16), 512-col halves in single PSUM banks ----
            for i in range(G):
                critic = opool.tile([128, FD], fp16, tag="critic")
                for h in range(2):
                    hs = slice(h * 512, (h + 1) * 512)
                    v1ps = psV.tile([128, 512], f32, tag="v1")
                    for k in range(4):
                        nc.tensor.matmul(v1ps[:], wv1[k], xhs[i][k][:, hs],
                                         start=(k == 0), stop=(k == 3))
                    v1 = opool.tile([128, 512], fp16, tag="v1s")
                    nc.vector.tensor_scalar(v1[:], v1ps[:], vt[:, 13:14], 0.0, Alu.add, Alu.max)
                    v2ps = psC.tile([128, 512], f32, tag="v2")
                    nc.tensor.matmul(v2ps[:], wv2, v1[:], start=True, stop=True)
                    nc.vector.tensor_scalar(critic[:, hs], v2ps[:], vt[:, 14:15], 0.0,
                                            Alu.add, Alu.max)
                nc.sync.dma_start(criticT[:, a0s[i]:a0s[i] + FD], critic[0:64, :])
                nc.sync.dma_start(criticT[:, a0s[i] + FD:a0s[i] + CHUNK], critic[64:128, :])

    nc.finalize()
    _BUILD_CACHE[bcore] = nc
    return nc


def _q(v, bits=CBITS):
    # truncate mantissa to `bits` bits (matches HW f32r operand truncation)
    v = np.asarray(v, np.float64)
    m, e = np.frexp(v)
    return np.ldexp(np.trunc(m * (1 << bits)) / (1 << bits), e)


def _chunks(W):
    """[64, 256] -> 4 lhsT chunks [128, 128]; chunk k: features (k%2)*128, batch-chunk k//2."""
    c = np.zeros((4, 128, 128), np.float64)
    c[0, :, 0:64] = W[:, 0:128].T
    c[1, :, 0:64] = W[:, 128:256].T
    c[2, :, 64:128] = W[:, 0:128].T
    c[3, :, 64:128] = W[:, 128:256].T
    return np.concatenate(list(c), axis=1)  # [128, 512]


def _blockdiag2(w64):
    out = np.zeros((128, 128), np.float64)
    out[0:64, 0:64] = w64
    out[64:128, 64:128] = w64
    return out


def _make_consts(W1, b1, W2, b2, Wo, bo, beta_in, thr_in, beta_out, Wv1, bv1, Wv2, bv2):
    W1 = np.asarray(W1, np.float64); b1 = np.asarray(b1, np.float64)
    W2 = np.asarray(W2, np.float64); b2 = np.asarray(b2, np.float64)
    Wo = np.asarray(Wo, np.float64); bo = np.asarray(bo, np.float64)
    Wv1 = np.asarray(Wv1, np.float64); bv1 = np.asarray(bv1, np.float64)
    Wv2 = np.asarray(Wv2, np.float64); bv2 = np.asarray(bv2, np.float64)
    beta = np.clip(np.asarray(beta_in, np.float64), 0.0, 1.0)
    thr = np.asarray(thr_in, np.float64)
    boc = float(np.clip(np.asarray(beta_out, np.float64), 0.0, 1.0)[0])
    it = 1.0 / thr

    st = lambda v: np.tile(np.asarray(v, np.float64), 2)  # stack [64] -> [128]
    betaS, thrS, itS = st(beta), st(thr), st(it)
    b1p = st(b1 * it)
    g_c = b1p + betaS - 1.5

    # fp16-pair W1/thr chunks
    W1s = _chunks(W1 * it[:, None])
    W1h = W1s.astype(np.float16)
    W1l = (W1s - W1h.astype(np.float64)).astype(np.float16)
    wv1c = _chunks(Wv1).astype(np.float16)
    wv2bd = _blockdiag2(Wv2.T).astype(np.float16)
    wp16 = np.concatenate(
        [W1h, W1l, wv1c, wv2bd.astype(np.float16)], axis=1).astype(np.float16)

    # per-step coefficients (host-rounded so the K-fold matches the HW values)
    cts = [_q(betaS ** (-float(t))) for t in range(1, 10)]
    c10 = _q(betaS ** (-10.0))

    # K tables
    Ks = []
    K = b1p - 1.0
    for t in range(1, 10):
        Ks.append(K.copy())         # K_t used by compare t
        K = K + cts[t - 1] * g_c
    Ks.append(K.copy())             # K_10
    W2bd = _blockdiag2(W2) * itS[:, None]
    rs2 = W2bd.sum(1)
    b2p = st(b2 * it)
    K11 = K + c10 * (b2p + betaS - 1.0 + rs2 / 2 - 0.5)
    Ks.append(K11)                  # K_11 (spk2 compare)
    L = boc * thrS * betaS ** 10.0
    Wobd = _blockdiag2(Wo)
    rso = Wobd.sum(1)
    boS = st(bo)
    bias_f = L * K11 + boc * thrS + rso / 2 + boS

    # f32r pack: 9 sigma-diags, acmb, bcmb  (lhsT[j, p] layout -> store transposed)
    packs = []
    for t in range(1, 10):
        packs.append(np.diag(-cts[t - 1] / 2.0))
    acmb_rows = c10[:, None] * (W2bd - np.eye(128)) / 2.0   # out[p] += sum_j acmb_rows[p,j] sg[j]
    packs.append(acmb_rows.T)
    bcmb_rows = Wobd / (2.0 * L[:, None])
    packs.append(bcmb_rows.T)
    wpr = _q(np.concatenate(packs, axis=1)).astype(np.float32)

    vecs = np.zeros((128, 16), np.float32)
    for i in range(11):
        vecs[:, i] = Ks[i]
    vecs[:, 11] = L
    vecs[:, 12] = bias_f
    vecs[:, 13] = st(bv1)
    vecs[:, 14] = st(bv2)
    cvec = np.zeros((128, 16), np.float32)
    for t in range(9):
        cvec[:, t] = cts[t]

    return dict(
        wp16=np.ascontiguousarray(wp16),
        wpr=np.ascontiguousarray(wpr),
        vecs=np.ascontiguousarray(vecs),
        cvec=np.ascontiguousarray(cvec),
    )


LAST_RESULT = None


def _run(x, consts, bcore):
    global LAST_RESULT
    nc = _build(bcore)
    n_cores = x.shape[0] // bcore
    xT = np.ascontiguousarray(x.T.astype(np.float32))  # [256, B]
    xh = xT.astype(np.float16)
    xl = (xT - xh.astype(np.float32)).astype(np.float16)
    in_maps = []
    for c in range(n_cores):
        m = dict(consts)
        m["xh"] = np.ascontiguousarray(xh[:, c * bcore:(c + 1) * bcore])
        m["xl"] = np.ascontiguousarray(xl[:, c * bcore:(c + 1) * bcore])
        in_maps.append(m)
    kw = {}
    if os.environ.get("KTRACE"):
        kw = dict(trace=True, tmpdir=os.environ.get("KTRACE_DIR") or None)
    res = run_bass_kernel_spmd(nc, in_maps, list(range(n_cores)), **kw)
    LAST_RESULT = res
    actorT = np.concatenate([r["actorT"] for r in res.results], axis=1)
    criticT = np.concatenate([r["criticT"] for r in res.results], axis=1)
    actor = np.ascontiguousarray(actorT.T.astype(np.float32) * np.float32(PI))
    critic = np.ascontiguousarray(criticT.T.astype(np.float32))
    return actor, critic


def kernel(x, W1, b1, W2, b2, Wo, bo, beta_in, thr_in, beta_out, Wv1, bv1, Wv2, bv2):
    x = np.asarray(x, np.float32)
    consts = _make_consts(W1, b1, W2, b2, Wo, bo, beta_in, thr_in, beta_out,
                          Wv1, bv1, Wv2, bv2)
    return _run(x, consts, B_CORE)
